# revision 22
# baseline (speedup 1.0000x reference)
"""Multi-head self-attention (B=2, T=2048, D=1024, H=16, causal) on 8 TRN2
NeuronCores.

Sharding: data parallel over batch (2) x tensor parallel over heads (4 groups
of 4 heads) = 8 cores. Each core computes qkv projection for its 4 heads, the
causal attention, and a partial out-projection over its heads' channels; the
host sums the 4 partials per batch and adds b_out.

Device layouts (per core):
  xT      [D=1024, T=2048]   x[b] transposed (host)
  wqkT    [D, 512]           q/k projection weights, chan order
                             [q(h0)|q(h1)] [k(h0)|k(h1)] [q(h2)|q(h3)] [k(h2)|k(h3)]
                             (64 rows each; q rows pre-scaled by 1/sqrt(HD))
  wvT     [D, 256]           v weights [v(h0)|v(h1)|v(h2)|v(h3)]
  woutT   [256, D]           W_out columns for this core's heads, transposed
  mask    [128, 128]         mask[r, c] = 1.0 if r <= c else 0 (causal, within-tile)
Output:
  outT    [D, T]             partial (pre-bias) out-projection, transposed

Attention per head: scores^T tiles [tk=128, tq=512] = kT.T @ qT (K=HD=64),
exp on ScalarE (scores are bounded, no max-subtraction needed), causal mask by
column-trimming + one 128x128 triangular mask multiply on the staircase block,
then out'^T [65, tq] = v_aug.T @ exp^T accumulated over tk tiles where v_aug
is v with a ones column appended - row 64 of the result is sum(exp), used to
normalize. All matmuls run as float32r (full PE rate at free dim >= 256).
"""

import numpy as np

import concourse.bass as bass
import concourse.tile as tile
from concourse import mybir
from concourse.bass_utils import run_bass_kernel_spmd

B, T, D, H = 2, 2048, 1024, 16
HD = D // H  # 64
NCORES = 8
HPC = 4  # heads per core
F32 = mybir.dt.float32
F32R = mybir.dt.float32r
EXP = mybir.ActivationFunctionType.Exp

_NTQ = T // 512  # 4 tq stripes of 512
_NTK = T // 128  # 16 tk tiles of 128
_NKD = D // 128  # 8 contraction tiles over D


def _apply_drain_patch():
    """This walrus build rejects >1 sync-wait command on a CTRL/Drain, so
    split the Tile tail-drain's waits across one drain instruction per
    pending proc."""
    import bass_rust

    if getattr(tile.TileContext, "_drain_patch_applied", False):
        return

    def _split_drain_and_barrier(self, tick_clock, wait_clock):
        nc = self.nc
        gc = tick_clock.global_clock
        NP = 27
        ticks = [gc[p] for p in range(NP)]
        for p in range(NP):
            if ticks[p] <= 0:
                continue
            partial = bass_rust.VectorClock(
                [ticks[q] if q == p else 0 for q in range(NP)]
            )
            d = nc.sync.drain()
            wait_clock.add_sem_waits(d.ins, bass_rust.ScopedClock({None: partial}))
        nc.all_engine_barrier()
        assert self.sems is not None
        popped = nc._tile_sem_poison_stack.pop()
        assert popped is self._sem_poison
        nc.clear_and_free_semaphores(list(self.sems.allocated().values()))
        nc.all_engine_barrier()

    tile.TileContext._drain_and_barrier = _split_drain_and_barrier
    tile.TileContext._drain_patch_applied = True


def _split_waits(nc):
    """This walrus build allows only one sync-wait command per instruction;
    move extra waits onto same-engine NOPs inserted right before."""
    import bass_rust

    f = nc.m.functions[0]
    ctr = 0
    for blk in f.blocks:
        insts = list(blk.instructions)
        new = []
        changed = False
        for inst in insts:
            si = getattr(inst, "sync_info", None)
            if si is not None and len(si.on_wait) > 1:
                waits = list(si.on_wait)
                for w in waits[:-1]:
                    nop = mybir.InstNoOp(name=f"wsplit-{ctr}", ins=[], outs=[])
                    ctr += 1
                    nop.engine = inst.engine
                    nop.sync_info = bass_rust.SyncInfo(on_wait=[w], on_update=[])
                    nc.register_instruction(nop, overwrite=True)
                    new.append(nop)
                inst.sync_info = bass_rust.SyncInfo(
                    on_wait=[waits[-1]], on_update=list(si.on_update))
                changed = True
            new.append(inst)
        if changed:
            blk.instructions = new


def build_nc():
    """Build the per-core Bass program (identical on all 8 cores)."""
    _apply_drain_patch()
    nc = bass.Bass("TRN2", target_bir_lowering=False, debug=False,
                   num_devices=NCORES)

    xT = nc.dram_tensor("xT", [D, T], F32R, kind="ExternalInput").ap()
    wqkT = nc.dram_tensor("wqkT", [D, 8 * HD], F32R, kind="ExternalInput").ap()
    bqk = nc.dram_tensor("bqk", [128, 4], F32, kind="ExternalInput").ap()
    wvT = nc.dram_tensor("wvT", [D, 4 * HD], F32R, kind="ExternalInput").ap()
    bv = nc.dram_tensor("bv", [1, 4 * HD], F32, kind="ExternalInput").ap()
    woutT = nc.dram_tensor("woutT", [4 * HD, D], F32R, kind="ExternalInput").ap()
    mask = nc.dram_tensor("mask", [128, 128], F32, kind="ExternalInput").ap()
    one = nc.dram_tensor("one", [1, 1], F32R, kind="ExternalInput").ap()
    outT = nc.dram_tensor("outT", [D, T], F32, kind="ExternalOutput").ap()

    with tile.TileContext(nc) as tc:
        _emit(nc, tc, xT, wqkT, bqk, wvT, bv, woutT, mask, one, outT)
    _split_waits(nc)
    return nc


def _emit(nc, tc, xT, wqkT, bqk, wvT, bv, woutT, mask, one, outT):
    import contextlib

    with contextlib.ExitStack() as ctx:
        const = ctx.enter_context(tc.tile_pool(name="const", bufs=1))
        persist = ctx.enter_context(tc.tile_pool(name="persist", bufs=1))

        wqk_sb = const.tile([128, _NKD, 8 * HD], F32R)
        wv_sb = const.tile([128, _NKD, 4 * HD], F32R)
        _wqk = wqkT.rearrange("(k p) c -> p k c", p=128)
        _wv = wvT.rearrange("(k p) c -> p k c", p=128)
        for k in range(_NKD):
            nc.sync.dma_start(out=wqk_sb[:, k, :], in_=_wqk[:, k, :])
            nc.sync.dma_start(out=wv_sb[:, k, :], in_=_wv[:, k, :])
        wo_sb = const.tile([128, 2, D], F32R)
        nc.sync.dma_start(out=wo_sb, in_=woutT.rearrange("(k p) c -> p k c", p=128))
        mask_sb = const.tile([128, 128], F32)
        nc.sync.dma_start(out=mask_sb, in_=mask)
        bqk_sb = const.tile([128, 4], F32)
        nc.sync.dma_start(out=bqk_sb, in_=bqk)
        bv_sb = const.tile([128, 4 * HD], F32)
        nc.gpsimd.dma_start(
            out=bv_sb,
            in_=bass.AP(tensor=bv.tensor, offset=bv.offset,
                        ap=[[0, 128], [1, 4 * HD]]),
        )

        # qkT[:, m, :]: m=0 -> q(h0)|q(h1), 1 -> k(h0)|k(h1), 2 -> q(h2)|q(h3),
        # 3 -> k(h2)|k(h3); partition p<64 is head h0/h2, p>=64 is h1/h3.
        qkT = persist.tile([128, 4, T], F32R)
        # v with a trailing ones column: [tq-part, tk-tile, head, HD+1]
        # (sum(exp) lands on psum partition 64)
        vaug = persist.tile([128, _NTK, HPC, HD + 1], F32R)
        nc.gpsimd.dma_start(
            out=vaug[:, :, :, HD:HD + 1],
            in_=bass.AP(tensor=one.tensor, offset=one.offset,
                        ap=[[0, 128], [0, _NTK * HPC], [0, 1]]),
        )
        # attention output^T, stacked [h0|h1] / [h2|h3] on partitions
        aT = persist.tile([128, 2, T], F32R)

        # ---- Phase A: qkv projections ----
        with tc.tile_pool(name="xp", bufs=1) as xp, \
             tc.tile_pool(name="psA", bufs=3, space="PSUM") as psA, \
             tc.tile_pool(name="psV", bufs=2, space="PSUM") as psV:
            xT_sb = xp.tile([128, _NKD, T], F32R)
            _xTr = xT.rearrange("(k p) t -> p k t", p=128)
            for k in range(_NKD):
                for n in range(_NTQ):
                    nc.sync.dma_start(out=xT_sb[:, k, n * 512:(n + 1) * 512],
                                      in_=_xTr[:, k, n * 512:(n + 1) * 512])
            for n in range(_NTQ):
                for m in range(4):
                    ps = psA.tile([128, 512], F32, tag="qk")
                    for k in range(_NKD):
                        nc.tensor.matmul(
                            ps,
                            lhsT=wqk_sb[:, k, m * 128:(m + 1) * 128],
                            rhs=xT_sb[:, k, n * 512:(n + 1) * 512],
                            start=(k == 0), stop=(k == _NKD - 1),
                        )
                    nc.vector.tensor_scalar_add(
                        out=qkT[:, m, n * 512:(n + 1) * 512],
                        in0=ps, scalar1=bqk_sb[:, m:m + 1],
                    )
            for t in range(_NTK):
                psv = psV.tile([128, 4 * HD], F32, tag="v")
                for k in range(_NKD):
                    nc.tensor.matmul(
                        psv,
                        lhsT=xT_sb[:, k, t * 128:(t + 1) * 128],
                        rhs=wv_sb[:, k, :],
                        start=(k == 0), stop=(k == _NKD - 1),
                    )
                nc.vector.tensor_add(
                    out=vaug[:, t, :, 0:HD],
                    in0=psv.rearrange("p (h d) -> p h d", h=HPC),
                    in1=bv_sb.rearrange("p (h d) -> p h d", h=HPC),
                )

        # ---- Phase B+C: attention per (stripe j, head h), then out_proj(j) ----
        with tc.tile_pool(name="psB", bufs=2, space="PSUM") as psB, \
             tc.tile_pool(name="expp", bufs=3) as expp, \
             tc.tile_pool(name="small", bufs=3) as small, \
             tc.tile_pool(name="dscr", bufs=2, space="DRAM") as dscr, \
             tc.tile_pool(name="outp", bufs=3) as outp:
            for j in range(_NTQ):
                ntk = 4 * j + 4
                ps_avs = []
                gath = small.tile([HPC, 512], F32, tag="gath")
                for h in range(HPC):
                    pair, sub = h // 2, h % 2
                    qT_h = qkT[sub * 64:(sub + 1) * 64, 2 * pair, :]
                    kT_h = qkT[sub * 64:(sub + 1) * 64, 2 * pair + 1, :]
                    qs = qT_h[:, j * 512:(j + 1) * 512]
                    ps_av = psB.tile([HD + 1, 512], F32, tag="av", bufs=4)
                    ps_avs.append(ps_av)
                    for i2 in range(0, ntk, 2):
                        a0, a1 = i2 - 4 * j, i2 + 1 - 4 * j
                        ps2 = psB.tile([128, 2, 512], F32, tag="s")
                        nc.tensor.matmul(
                            ps2[:, 0, :], lhsT=kT_h[:, i2 * 128:(i2 + 1) * 128],
                            rhs=qs, start=True, stop=True)
                        nc.tensor.matmul(
                            ps2[:, 1, :], lhsT=kT_h[:, (i2 + 1) * 128:(i2 + 2) * 128],
                            rhs=qs, start=True, stop=True)
                        expT = expp.tile([128, 2, 512], F32R, tag="e")
                        if a1 < 0:
                            # both tiles fully below the diagonal
                            nc.scalar.activation(expT[:, 0, :], ps2[:, 0, :], EXP)
                            nc.scalar.activation(expT[:, 1, :], ps2[:, 1, :], EXP)
                            c0 = c1 = 0
                        else:
                            c0 = max(a0, 0) * 128
                            nc.scalar.activation(
                                expT[:, 0, c0:512], ps2[:, 0, c0:512], EXP)
                            if a0 >= 0:
                                nc.vector.tensor_mul(
                                    expT[:, 0, c0:c0 + 128],
                                    expT[:, 0, c0:c0 + 128], mask_sb)
                            c1 = max(a1, 0) * 128
                            nc.scalar.activation(
                                expT[:, 1, c1:512], ps2[:, 1, c1:512], EXP)
                            nc.vector.tensor_mul(
                                expT[:, 1, c1:c1 + 128],
                                expT[:, 1, c1:c1 + 128], mask_sb)
                        nc.tensor.matmul(
                            ps_av[:, c0:512], lhsT=vaug[:, i2, h, :],
                            rhs=expT[:, 0, c0:512],
                            start=(i2 == 0), stop=False)
                        nc.tensor.matmul(
                            ps_av[:, c1:512], lhsT=vaug[:, i2 + 1, h, :],
                            rhs=expT[:, 1, c1:512],
                            start=False, stop=(i2 + 2 == ntk))
                    # stage this head's sum(exp) row (psum row 64) into the
                    # gather tile at partition h (DMA shifts partitions)
                    srow = small.tile([HD + 1, 512], F32, tag="sr")
                    nc.vector.tensor_copy(srow[HD:HD + 1, :], ps_av[HD:HD + 1, :])
                    nc.sync.dma_start(out=gath[h:h + 1, :], in_=srow[HD:HD + 1, :])
                # one reciprocal for all 4 heads (DVE cost is free-size bound)
                rec4 = small.tile([HPC, 512], F32, tag="rec")
                nc.vector.reciprocal(rec4, gath)
                dram4 = dscr.tile([HPC, 512], F32, tag="dt")
                nc.sync.dma_start(out=dram4, in_=rec4)
                for h in range(HPC):
                    pair, sub = h // 2, h % 2
                    sl = dram4[h:h + 1, :]
                    rb = small.tile([HD, 512], F32, tag="rb", bufs=4)
                    nc.gpsimd.dma_start(
                        out=rb,
                        in_=bass.AP(tensor=sl.tensor, offset=sl.offset,
                                    ap=[[0, HD]] + [list(p) for p in sl.ap[1:]]),
                    )
                    tmp = small.tile([HD, 512], F32R, tag="tmp", bufs=4)
                    nc.vector.tensor_mul(tmp, ps_avs[h][0:HD, :], rb)
                    nc.sync.dma_start(
                        out=aT[sub * 64:(sub + 1) * 64, pair,
                               j * 512:(j + 1) * 512],
                        in_=tmp)
                for m in range(D // 128):
                    po = psB.tile([128, 2, 512], F32, tag="s")
                    for kk in range(2):
                        nc.tensor.matmul(
                            po[:, 0, :],
                            lhsT=wo_sb[:, kk, m * 128:(m + 1) * 128],
                            rhs=aT[:, kk, j * 512:(j + 1) * 512],
                            start=(kk == 0), stop=(kk == 1),
                        )
                    ot = outp.tile([128, 512], F32, tag="ot")
                    nc.vector.tensor_copy(ot, po[:, 0, :])
                    nc.sync.dma_start(
                        out=outT[m * 128:(m + 1) * 128, j * 512:(j + 1) * 512],
                        in_=ot)


def shard_inputs(x, W_qkv, b_qkv, W_out):
    """Host-side packing: one input dict per core."""
    x = np.asarray(x, np.float32)
    Wr = np.asarray(W_qkv, np.float32).reshape(H, 3, HD, D)
    br = np.asarray(b_qkv, np.float32).reshape(H, 3, HD)
    W_out = np.asarray(W_out, np.float32)
    scale = 1.0 / np.sqrt(HD)

    mask128 = np.triu(np.ones((128, 128), np.float32))
    in_maps = []
    for c in range(NCORES):
        b, g = divmod(c, 4)
        hh = [4 * g + i for i in range(HPC)]
        # chan-tile order: q(h0)|q(h1), k(h0)|k(h1), q(h2)|q(h3), k(h2)|k(h3)
        qk_rows, qk_bias = [], []
        for p in range(2):
            h0, h1 = hh[2 * p], hh[2 * p + 1]
            qk_rows += [Wr[h0, 0] * scale, Wr[h1, 0] * scale, Wr[h0, 1], Wr[h1, 1]]
            qk_bias += [br[h0, 0] * scale, br[h1, 0] * scale, br[h0, 1], br[h1, 1]]
        wqk = np.concatenate(qk_rows, 0)          # [512, D]
        bqk = np.concatenate(qk_bias, 0)          # [512]
        wv = np.concatenate([Wr[h, 2] for h in hh], 0)   # [256, D]
        bvv = np.concatenate([br[h, 2] for h in hh], 0)  # [256]
        cols = np.concatenate([np.arange(h * HD, (h + 1) * HD) for h in hh])
        in_maps.append({
            "xT": np.ascontiguousarray(x[b].T),
            "wqkT": np.ascontiguousarray(wqk.T),
            "bqk": np.ascontiguousarray(bqk.reshape(4, 128).T),
            "wvT": np.ascontiguousarray(wv.T),
            "bv": np.ascontiguousarray(bvv.reshape(1, 4 * HD)),
            "woutT": np.ascontiguousarray(W_out[:, cols].T),
            "mask": mask128,
            "one": np.ones((1, 1), np.float32),
        })
    return in_maps


_NC = None


def kernel(x, mask, W_qkv, b_qkv, W_out, b_out, **run_kwargs):
    global _NC
    if _NC is None:
        _NC = build_nc()
    in_maps = shard_inputs(x, W_qkv, b_qkv, W_out)
    res = run_bass_kernel_spmd(_NC, in_maps, core_ids=list(range(NCORES)),
                               **run_kwargs)
    b_out = np.asarray(b_out, np.float64)
    outs = []
    for b in range(B):
        acc = np.zeros((D, T), np.float64)
        for g in range(4):
            acc += res.results[4 * b + g]["outT"]
        outs.append(acc.T + b_out[None, :])
    out = np.stack(outs).astype(np.float32)
    if run_kwargs:
        kernel.last_results = res
    return out


# revision 23
# speedup vs baseline: 1.0878x; 1.0878x over previous
"""Multi-head self-attention (B=2, T=2048, D=1024, H=16, causal) on 8 TRN2
NeuronCores.

Sharding: data parallel over batch (2) x tensor parallel over heads (4 groups
of 4 heads) = 8 cores. Each core computes the qkv projection for its 4 heads,
causal attention, and a partial out-projection over its heads' channels; the
host sums the 4 partials per batch and adds b_out.

Precision: x / projection weights are bf16 (input DMA + FWL weight loads),
q/k activations and the scores matmul are float32r (exp input accuracy),
exp output / v / attention-output path are bf16, all psum accumulation fp32.

Device inputs per core (host-prearranged, fully contiguous):
  xr    [128, 8, 2048] bf16   x[b]^T tiled (p, k, t): xr[p,k,t] = x[b][t, 128k+p]
  wqk   [128, 8, 512]  bf16   q/k weights, chan order
                              [q(h0)|q(h1)] [k(h0)|k(h1)] [q(h2)|q(h3)] [k(h2)|k(h3)]
                              (64 each; q pre-scaled by 1/sqrt(HD))
  wv    [128, 8, 256]  bf16   v weights [v(h0)|v(h1)|v(h2)|v(h3)]
  wo    [128, 2, 1024] bf16   W_out columns for these heads, transposed+tiled
  bqk   [128, 4] f32, bv [1, 256] f32, mask [128,128] bf16, one [1,1] bf16
Output:
  outT  [D, T] f32            partial (pre-bias) out-projection, transposed

Attention per head: scores^T tiles [tk=128, tq=512] = kT.T @ qT (K=HD=64,
f32r); the two heads of a pair sit at partitions 0:64 / 64:128 so their
scores matmuls land in disjoint PE row-groups and run concurrently. exp on
ScalarE (scores are bounded, no max-subtraction), causal handling by column
trimming + one triangular 128x128 mask multiply on the staircase block, then
out'^T [65, tq] = v_aug.T @ exp^T accumulated over tk tiles (v_aug has a
trailing ones column, so psum row 64 is sum(exp)). The 4 heads' sum(exp)
rows are gathered onto partitions 0..3 of one tile for a single batched
DVE reciprocal per stripe.
"""

import numpy as np
import ml_dtypes

import concourse.bass as bass
import concourse.tile as tile
from concourse import mybir
from concourse.bass_utils import run_bass_kernel_spmd

B, T, D, H = 2, 2048, 1024, 16
HD = D // H  # 64
NCORES = 8
HPC = 4  # heads per core
F32 = mybir.dt.float32
F32R = mybir.dt.float32r
BF16 = mybir.dt.bfloat16
EXP = mybir.ActivationFunctionType.Exp
BF16NP = ml_dtypes.bfloat16

_NTQ = T // 512  # 4 tq stripes of 512
_NTK = T // 128  # 16 tk tiles of 128
_NKD = D // 128  # 8 contraction tiles over D


def _apply_drain_patch():
    """This walrus build rejects >1 sync-wait command on a CTRL/Drain, so
    split the Tile tail-drain's waits across one drain instruction per
    pending proc."""
    import bass_rust

    if getattr(tile.TileContext, "_drain_patch_applied", False):
        return

    def _split_drain_and_barrier(self, tick_clock, wait_clock):
        nc = self.nc
        gc = tick_clock.global_clock
        NP = 27
        ticks = [gc[p] for p in range(NP)]
        for p in range(NP):
            if ticks[p] <= 0:
                continue
            partial = bass_rust.VectorClock(
                [ticks[q] if q == p else 0 for q in range(NP)]
            )
            d = nc.sync.drain()
            wait_clock.add_sem_waits(d.ins, bass_rust.ScopedClock({None: partial}))
        nc.all_engine_barrier()
        assert self.sems is not None
        popped = nc._tile_sem_poison_stack.pop()
        assert popped is self._sem_poison
        nc.clear_and_free_semaphores(list(self.sems.allocated().values()))
        nc.all_engine_barrier()

    tile.TileContext._drain_and_barrier = _split_drain_and_barrier
    tile.TileContext._drain_patch_applied = True


def _split_waits(nc):
    """This walrus build allows only one sync-wait command per instruction;
    move extra waits onto same-engine NOPs inserted right before."""
    import bass_rust

    f = nc.m.functions[0]
    ctr = 0
    for blk in f.blocks:
        insts = list(blk.instructions)
        new = []
        changed = False
        for inst in insts:
            si = getattr(inst, "sync_info", None)
            if si is not None and len(si.on_wait) > 1:
                waits = list(si.on_wait)
                for w in waits[:-1]:
                    nop = mybir.InstNoOp(name=f"wsplit-{ctr}", ins=[], outs=[])
                    ctr += 1
                    nop.engine = inst.engine
                    nop.sync_info = bass_rust.SyncInfo(on_wait=[w], on_update=[])
                    nc.register_instruction(nop, overwrite=True)
                    new.append(nop)
                inst.sync_info = bass_rust.SyncInfo(
                    on_wait=[waits[-1]], on_update=list(si.on_update))
                changed = True
            new.append(inst)
        if changed:
            blk.instructions = new


def build_nc():
    """Build the per-core Bass program (identical on all 8 cores)."""
    _apply_drain_patch()
    nc = bass.Bass("TRN2", target_bir_lowering=False, debug=False,
                   num_devices=NCORES)

    xr = nc.dram_tensor("xr", [128, _NKD, T], BF16, kind="ExternalInput").ap()
    wqk = nc.dram_tensor("wqk", [128, _NKD, 8 * HD], BF16,
                         kind="ExternalInput").ap()
    bqk = nc.dram_tensor("bqk", [128, 4], F32, kind="ExternalInput").ap()
    wv = nc.dram_tensor("wv", [128, _NKD, 4 * HD], BF16,
                        kind="ExternalInput").ap()
    bv = nc.dram_tensor("bv", [1, 4 * HD], F32, kind="ExternalInput").ap()
    wo = nc.dram_tensor("wo", [128, 2, D], BF16, kind="ExternalInput").ap()
    mask = nc.dram_tensor("mask", [128, 128], BF16, kind="ExternalInput").ap()
    one = nc.dram_tensor("one", [1, 1], BF16, kind="ExternalInput").ap()
    outT = nc.dram_tensor("outT", [D, T], F32, kind="ExternalOutput").ap()

    with tile.TileContext(nc) as tc:
        _emit(nc, tc, xr, wqk, bqk, wv, bv, wo, mask, one, outT)
    _split_waits(nc)
    return nc


def _emit(nc, tc, xr, wqk, bqk, wv, bv, wo, mask, one, outT):
    import contextlib

    with contextlib.ExitStack() as ctx:
        const = ctx.enter_context(tc.tile_pool(name="const", bufs=1))
        persist = ctx.enter_context(tc.tile_pool(name="persist", bufs=1))

        # weight loads on the Scalar HWDGE queue (idle at kernel start);
        # x on the Sync queue; small constants via gpsimd SWDGE
        wqk_sb = const.tile([128, _NKD, 8 * HD], BF16)
        wv_sb = const.tile([128, _NKD, 4 * HD], BF16)
        wo_sb = const.tile([128, 2, D], BF16)
        nc.scalar.dma_start(out=wqk_sb, in_=wqk)
        nc.scalar.dma_start(out=wv_sb, in_=wv)
        nc.scalar.dma_start(out=wo_sb, in_=wo)
        mask_sb = const.tile([128, 128], BF16)
        nc.gpsimd.dma_start(out=mask_sb, in_=mask)
        bqk_sb = const.tile([128, 4], F32)
        nc.gpsimd.dma_start(out=bqk_sb, in_=bqk)
        bv_sb = const.tile([128, 4 * HD], F32)
        nc.gpsimd.dma_start(
            out=bv_sb,
            in_=bass.AP(tensor=bv.tensor, offset=bv.offset,
                        ap=[[0, 128], [1, 4 * HD]]),
        )

        # qkT[:, m, :]: m=0 -> q(h0)|q(h1), 1 -> k(h0)|k(h1), 2 -> q(h2)|q(h3),
        # 3 -> k(h2)|k(h3); partition p<64 is head h0/h2, p>=64 is h1/h3.
        qkT = persist.tile([128, 4, T], F32R)
        # v with a trailing ones column: [tq-part, tk-tile, head, HD+1]
        # (sum(exp) lands on psum partition 64)
        vaug = persist.tile([128, _NTK, HPC, HD + 1], BF16)
        nc.gpsimd.dma_start(
            out=vaug[:, :, :, HD:HD + 1],
            in_=bass.AP(tensor=one.tensor, offset=one.offset,
                        ap=[[0, 128], [0, _NTK * HPC], [0, 1]]),
        )
        # attention output^T, stacked [h0|h1] / [h2|h3] on partitions
        aT = persist.tile([128, 2, T], BF16)

        # ---- Phase A: qkv projections ----
        with tc.tile_pool(name="xp", bufs=1) as xp, \
             tc.tile_pool(name="psA", bufs=3, space="PSUM") as psA, \
             tc.tile_pool(name="psV", bufs=2, space="PSUM") as psV:
            xT_sb = xp.tile([128, _NKD, T], BF16)
            for n in range(_NTQ):
                nc.sync.dma_start(out=xT_sb[:, :, n * 512:(n + 1) * 512],
                                  in_=xr[:, :, n * 512:(n + 1) * 512])
            for n in range(_NTQ):
                for m in range(4):
                    ps = psA.tile([128, 512], F32, tag="qk")
                    for k in range(_NKD):
                        nc.tensor.matmul(
                            ps,
                            lhsT=wqk_sb[:, k, m * 128:(m + 1) * 128],
                            rhs=xT_sb[:, k, n * 512:(n + 1) * 512],
                            start=(k == 0), stop=(k == _NKD - 1),
                        )
                    nc.vector.tensor_scalar_add(
                        out=qkT[:, m, n * 512:(n + 1) * 512],
                        in0=ps, scalar1=bqk_sb[:, m:m + 1],
                    )
                for t in range(4 * n, 4 * n + 4):
                    psv = psV.tile([128, 4 * HD], F32, tag="v")
                    for k in range(_NKD):
                        nc.tensor.matmul(
                            psv,
                            lhsT=xT_sb[:, k, t * 128:(t + 1) * 128],
                            rhs=wv_sb[:, k, :],
                            start=(k == 0), stop=(k == _NKD - 1),
                        )
                    nc.vector.tensor_add(
                        out=vaug[:, t, :, 0:HD],
                        in0=psv.rearrange("p (h d) -> p h d", h=HPC),
                        in1=bv_sb.rearrange("p (h d) -> p h d", h=HPC),
                    )

        # ---- Phase B+C: attention (head pairs interleaved), out_proj(j) ----
        with tc.tile_pool(name="psB", bufs=4, space="PSUM") as psB, \
             tc.tile_pool(name="expp", bufs=6) as expp, \
             tc.tile_pool(name="small", bufs=3) as small, \
             tc.tile_pool(name="dscr", bufs=2, space="DRAM") as dscr, \
             tc.tile_pool(name="outp", bufs=3) as outp:
            for j in range(_NTQ):
                ntk = 4 * j + 4
                ps_avs = []
                gath = small.tile([HPC, 512], F32, tag="gath")
                for hp in range(2):
                    h0, h1 = 2 * hp, 2 * hp + 1
                    qT0 = qkT[0:64, 2 * hp, :]
                    kT0 = qkT[0:64, 2 * hp + 1, :]
                    qT1 = qkT[64:128, 2 * hp, :]
                    kT1 = qkT[64:128, 2 * hp + 1, :]
                    av0 = psB.tile([HD + 1, 512], F32, tag="av", bufs=4)
                    av1 = psB.tile([HD + 1, 512], F32, tag="av", bufs=4)
                    ps_avs += [av0, av1]
                    for i in range(ntk):
                        a = i - 4 * j  # >= 0 on the causal staircase
                        col0 = max(a, 0) * 128
                        # keep the f32r scores matmul free dim >= 256
                        c0s = min(col0, 256)
                        tk = slice(i * 128, (i + 1) * 128)
                        tq = slice(j * 512 + c0s, (j + 1) * 512)
                        s0 = psB.tile([128, 512], F32, tag="s", bufs=4)
                        s1 = psB.tile([128, 512], F32, tag="s", bufs=4)
                        # the two heads hit disjoint PE row groups (0:64 /
                        # 64:128) and run concurrently
                        nc.tensor.matmul(s0[:, c0s:512], lhsT=kT0[:, tk],
                                         rhs=qT0[:, tq], start=True, stop=True)
                        nc.tensor.matmul(s1[:, c0s:512], lhsT=kT1[:, tk],
                                         rhs=qT1[:, tq], start=True, stop=True)
                        e0 = expp.tile([128, 512], BF16, tag="e")
                        e1 = expp.tile([128, 512], BF16, tag="e")
                        nc.scalar.activation(e0[:, col0:512], s0[:, col0:512], EXP)
                        nc.scalar.activation(e1[:, col0:512], s1[:, col0:512], EXP)
                        if a >= 0:
                            nc.vector.tensor_mul(
                                e0[:, col0:col0 + 128],
                                e0[:, col0:col0 + 128], mask_sb)
                            nc.vector.tensor_mul(
                                e1[:, col0:col0 + 128],
                                e1[:, col0:col0 + 128], mask_sb)
                        nc.tensor.matmul(
                            av0[:, col0:512], lhsT=vaug[:, i, h0, :],
                            rhs=e0[:, col0:512],
                            start=(i == 0), stop=(i == ntk - 1))
                        nc.tensor.matmul(
                            av1[:, col0:512], lhsT=vaug[:, i, h1, :],
                            rhs=e1[:, col0:512],
                            start=(i == 0), stop=(i == ntk - 1))
                    for h, av in ((h0, av0), (h1, av1)):
                        # stage this head's sum(exp) row (psum row 64) into
                        # the gather tile at partition h (DMA shifts parts)
                        srow = small.tile([HD + 1, 512], F32, tag="sr", bufs=4)
                        nc.vector.tensor_copy(srow[HD:HD + 1, :],
                                              av[HD:HD + 1, :])
                        nc.sync.dma_start(out=gath[h:h + 1, :],
                                          in_=srow[HD:HD + 1, :])
                # one reciprocal for all 4 heads (DVE cost is free-size bound)
                rec4 = small.tile([HPC, 512], F32, tag="rec")
                nc.vector.reciprocal(rec4, gath)
                dram4 = dscr.tile([HPC, 512], F32, tag="dt")
                nc.sync.dma_start(out=dram4, in_=rec4)
                for h in range(HPC):
                    pair, sub = h // 2, h % 2
                    sl = dram4[h:h + 1, :]
                    rb = small.tile([HD, 512], F32, tag="rb", bufs=4)
                    nc.gpsimd.dma_start(
                        out=rb,
                        in_=bass.AP(tensor=sl.tensor, offset=sl.offset,
                                    ap=[[0, HD]] + [list(p) for p in sl.ap[1:]]),
                    )
                    tmp = small.tile([HD, 512], BF16, tag="tmp", bufs=4)
                    nc.vector.tensor_mul(tmp, ps_avs[h][0:HD, :], rb)
                    nc.sync.dma_start(
                        out=aT[sub * 64:(sub + 1) * 64, pair,
                               j * 512:(j + 1) * 512],
                        in_=tmp)
                for m in range(D // 128):
                    po = psB.tile([128, 512], F32, tag="s", bufs=4)
                    for kk in range(2):
                        nc.tensor.matmul(
                            po,
                            lhsT=wo_sb[:, kk, m * 128:(m + 1) * 128],
                            rhs=aT[:, kk, j * 512:(j + 1) * 512],
                            start=(kk == 0), stop=(kk == 1),
                        )
                    ot = outp.tile([128, 512], F32, tag="ot")
                    nc.vector.tensor_copy(ot, po)
                    nc.scalar.dma_start(
                        out=outT[m * 128:(m + 1) * 128, j * 512:(j + 1) * 512],
                        in_=ot)


def shard_inputs(x, W_qkv, b_qkv, W_out):
    """Host-side packing: one input dict per core."""
    x = np.asarray(x, np.float32)
    Wr = np.asarray(W_qkv, np.float32).reshape(H, 3, HD, D)
    br = np.asarray(b_qkv, np.float32).reshape(H, 3, HD)
    W_out = np.asarray(W_out, np.float32)
    scale = 1.0 / np.sqrt(HD)

    def tile_pkc(a):
        # [R, C] with R = 128*k -> [128, k, C] contiguous bf16
        r, c = a.shape
        return np.ascontiguousarray(
            a.reshape(r // 128, 128, c).transpose(1, 0, 2).astype(BF16NP))

    mask128 = np.triu(np.ones((128, 128), BF16NP))
    in_maps = []
    for core in range(NCORES):
        b, g = divmod(core, 4)
        hh = [4 * g + i for i in range(HPC)]
        # chan-tile order: q(h0)|q(h1), k(h0)|k(h1), q(h2)|q(h3), k(h2)|k(h3)
        qk_rows, qk_bias = [], []
        for p in range(2):
            h0, h1 = hh[2 * p], hh[2 * p + 1]
            qk_rows += [Wr[h0, 0] * scale, Wr[h1, 0] * scale, Wr[h0, 1], Wr[h1, 1]]
            qk_bias += [br[h0, 0] * scale, br[h1, 0] * scale, br[h0, 1], br[h1, 1]]
        wqk = np.concatenate(qk_rows, 0)          # [512, D]
        bqk = np.concatenate(qk_bias, 0)          # [512]
        wv = np.concatenate([Wr[h, 2] for h in hh], 0)   # [256, D]
        bvv = np.concatenate([br[h, 2] for h in hh], 0)  # [256]
        cols = np.concatenate([np.arange(h * HD, (h + 1) * HD) for h in hh])
        in_maps.append({
            "xr": tile_pkc(np.ascontiguousarray(x[b].T)),
            "wqk": tile_pkc(np.ascontiguousarray(wqk.T)),
            "bqk": np.ascontiguousarray(bqk.reshape(4, 128).T),
            "wv": tile_pkc(np.ascontiguousarray(wv.T)),
            "bv": np.ascontiguousarray(bvv.reshape(1, 4 * HD)),
            "wo": tile_pkc(np.ascontiguousarray(W_out[:, cols].T)),
            "mask": mask128,
            "one": np.ones((1, 1), BF16NP),
        })
    return in_maps


_NC = None


def kernel(x, mask, W_qkv, b_qkv, W_out, b_out, **run_kwargs):
    global _NC
    if _NC is None:
        _NC = build_nc()
    in_maps = shard_inputs(x, W_qkv, b_qkv, W_out)
    res = run_bass_kernel_spmd(_NC, in_maps, core_ids=list(range(NCORES)),
                               **run_kwargs)
    b_out = np.asarray(b_out, np.float64)
    outs = []
    for b in range(B):
        acc = np.zeros((D, T), np.float64)
        for g in range(4):
            acc += res.results[4 * b + g]["outT"]
        outs.append(acc.T + b_out[None, :])
    out = np.stack(outs).astype(np.float32)
    if run_kwargs:
        kernel.last_results = res
    return out


# revision 25
# speedup vs baseline: 1.4461x; 1.3293x over previous
"""Multi-head self-attention (B=2, T=2048, D=1024, H=16, causal) on 8 TRN2
NeuronCores.

Sharding: data parallel over batch (2) x tensor parallel over heads (4 groups
of 4 heads) = 8 cores. Each core computes the qkv projection for its 4 heads,
causal attention, and a partial out-projection over its heads' channels; the
host sums the 4 partials per batch and adds b_out.

Precision: x / projection weights are bf16 (input DMA + FWL weight loads),
q/k activations and the scores matmul are float32r (exp input accuracy),
exp output / v / attention-output path are bf16, all psum accumulation fp32.

Device inputs per core (host-prearranged, fully contiguous):
  xr    [128, 8, 2048] bf16   x[b]^T tiled (p, k, t): xr[p,k,t] = x[b][t, 128k+p]
  wqk   [128, 8, 512]  bf16   q/k weights, chan order
                              [q(h0)|q(h1)] [k(h0)|k(h1)] [q(h2)|q(h3)] [k(h2)|k(h3)]
                              (64 each; q pre-scaled by 1/sqrt(HD))
  wv    [128, 8, 256]  bf16   v weights [v(h0)|v(h1)|v(h2)|v(h3)]
  wo    [128, 2, 1024] bf16   W_out columns for these heads, transposed+tiled
  bqk   [128, 4] f32, bv [1, 256] f32, mask [128,128] bf16, one [1,1] bf16
Output:
  outT  [D, T] f32            partial (pre-bias) out-projection, transposed

Attention per head: scores^T tiles [tk=128, tq=512] = kT.T @ qT (K=HD=64,
f32r); the two heads of a pair sit at partitions 0:64 / 64:128 so their
scores matmuls land in disjoint PE row-groups and run concurrently. exp on
ScalarE (scores are bounded, no max-subtraction), causal handling by column
trimming + one triangular 128x128 mask multiply on the staircase block, then
out'^T [65, tq] = v_aug.T @ exp^T accumulated over tk tiles (v_aug has a
trailing ones column, so psum row 64 is sum(exp)). The 4 heads' sum(exp)
rows are gathered onto partitions 0..3 of one tile for a single batched
DVE reciprocal per stripe.
"""

import numpy as np
import ml_dtypes

import concourse.bass as bass
import concourse.tile as tile
from concourse import mybir
from concourse.bass_utils import run_bass_kernel_spmd

B, T, D, H = 2, 2048, 1024, 16
HD = D // H  # 64
NCORES = 8
HPC = 4  # heads per core
F32 = mybir.dt.float32
F32R = mybir.dt.float32r
BF16 = mybir.dt.bfloat16
EXP = mybir.ActivationFunctionType.Exp
BF16NP = ml_dtypes.bfloat16

_NTQ = T // 512  # 4 tq stripes of 512
_NTK = T // 128  # 16 tk tiles of 128
_NKD = D // 128  # 8 contraction tiles over D


def _apply_drain_patch():
    """This walrus build rejects >1 sync-wait command on a CTRL/Drain, so
    split the Tile tail-drain's waits across one drain instruction per
    pending proc."""
    import bass_rust

    if getattr(tile.TileContext, "_drain_patch_applied", False):
        return

    def _split_drain_and_barrier(self, tick_clock, wait_clock):
        nc = self.nc
        gc = tick_clock.global_clock
        NP = 27
        ticks = [gc[p] for p in range(NP)]
        for p in range(NP):
            if ticks[p] <= 0:
                continue
            partial = bass_rust.VectorClock(
                [ticks[q] if q == p else 0 for q in range(NP)]
            )
            d = nc.sync.drain()
            wait_clock.add_sem_waits(d.ins, bass_rust.ScopedClock({None: partial}))
        nc.all_engine_barrier()
        assert self.sems is not None
        popped = nc._tile_sem_poison_stack.pop()
        assert popped is self._sem_poison
        nc.clear_and_free_semaphores(list(self.sems.allocated().values()))
        nc.all_engine_barrier()

    tile.TileContext._drain_and_barrier = _split_drain_and_barrier
    tile.TileContext._drain_patch_applied = True


def _split_waits(nc):
    """This walrus build allows only one sync-wait command per instruction;
    move extra waits onto same-engine NOPs inserted right before."""
    import bass_rust

    f = nc.m.functions[0]
    ctr = 0
    for blk in f.blocks:
        insts = list(blk.instructions)
        new = []
        changed = False
        for inst in insts:
            si = getattr(inst, "sync_info", None)
            if si is not None and len(si.on_wait) > 1:
                waits = list(si.on_wait)
                for w in waits[:-1]:
                    nop = mybir.InstNoOp(name=f"wsplit-{ctr}", ins=[], outs=[])
                    ctr += 1
                    nop.engine = inst.engine
                    nop.sync_info = bass_rust.SyncInfo(on_wait=[w], on_update=[])
                    nc.register_instruction(nop, overwrite=True)
                    new.append(nop)
                inst.sync_info = bass_rust.SyncInfo(
                    on_wait=[waits[-1]], on_update=list(si.on_update))
                changed = True
            new.append(inst)
        if changed:
            blk.instructions = new


def build_nc():
    """Build the per-core Bass program (identical on all 8 cores)."""
    _apply_drain_patch()
    nc = bass.Bass("TRN2", target_bir_lowering=False, debug=False,
                   num_devices=NCORES)

    xr = nc.dram_tensor("xr", [128, _NKD, T], BF16, kind="ExternalInput").ap()
    wqk = nc.dram_tensor("wqk", [128, _NKD, 8 * HD], BF16,
                         kind="ExternalInput").ap()
    bqk = nc.dram_tensor("bqk", [128, 4], F32, kind="ExternalInput").ap()
    wv = nc.dram_tensor("wv", [128, _NKD, 4 * HD], BF16,
                        kind="ExternalInput").ap()
    bv = nc.dram_tensor("bv", [1, 4 * HD], F32, kind="ExternalInput").ap()
    wo = nc.dram_tensor("wo", [128, 2, D], BF16, kind="ExternalInput").ap()
    mask = nc.dram_tensor("mask", [128, 128], BF16, kind="ExternalInput").ap()
    one = nc.dram_tensor("one", [1, 1], BF16, kind="ExternalInput").ap()
    outT = nc.dram_tensor("outT", [D, T], F32, kind="ExternalOutput").ap()

    with tile.TileContext(nc) as tc:
        _emit(nc, tc, xr, wqk, bqk, wv, bv, wo, mask, one, outT)
    _split_waits(nc)
    return nc


def _emit(nc, tc, xr, wqk, bqk, wv, bv, wo, mask, one, outT):
    import contextlib

    with contextlib.ExitStack() as ctx:
        const = ctx.enter_context(tc.tile_pool(name="const", bufs=1))
        persist = ctx.enter_context(tc.tile_pool(name="persist", bufs=1))

        # weight loads on the Scalar HWDGE queue (idle at kernel start);
        # x on the Sync queue; small constants via gpsimd SWDGE
        wqk_sb = const.tile([128, _NKD, 8 * HD], BF16)
        wv_sb = const.tile([128, _NKD, 4 * HD], BF16)
        wo_sb = const.tile([128, 2, D], BF16)
        nc.scalar.dma_start(out=wqk_sb, in_=wqk)
        nc.scalar.dma_start(out=wv_sb, in_=wv)
        nc.scalar.dma_start(out=wo_sb, in_=wo)
        mask_sb = const.tile([128, 128], BF16)
        nc.gpsimd.dma_start(out=mask_sb, in_=mask)
        bqk_sb = const.tile([128, 4], F32)
        nc.gpsimd.dma_start(out=bqk_sb, in_=bqk)
        bv_sb = const.tile([128, 4 * HD], F32)
        nc.gpsimd.dma_start(
            out=bv_sb,
            in_=bass.AP(tensor=bv.tensor, offset=bv.offset,
                        ap=[[0, 128], [1, 4 * HD]]),
        )

        # qkT[:, m, :]: m=0 -> q(h0)|q(h1), 1 -> k(h0)|k(h1), 2 -> q(h2)|q(h3),
        # 3 -> k(h2)|k(h3); partition p<64 is head h0/h2, p>=64 is h1/h3.
        qkT = persist.tile([128, 4, T], F32R)
        # v with a trailing ones column: [tq-part, tk-tile, head, HD+1]
        # (sum(exp) lands on psum partition 64)
        vaug = persist.tile([128, _NTK, HPC, HD + 1], BF16)
        nc.gpsimd.dma_start(
            out=vaug[:, :, :, HD:HD + 1],
            in_=bass.AP(tensor=one.tensor, offset=one.offset,
                        ap=[[0, 128], [0, _NTK * HPC], [0, 1]]),
        )
        # attention output^T, stacked [h0|h1] / [h2|h3] on partitions
        aT = persist.tile([128, 2, T], BF16)

        # ---- Phase A: qkv projections ----
        with tc.tile_pool(name="xp", bufs=1) as xp, \
             tc.tile_pool(name="psA", bufs=3, space="PSUM") as psA, \
             tc.tile_pool(name="psV", bufs=2, space="PSUM") as psV:
            xT_sb = xp.tile([128, _NKD, T], BF16)
            for k in range(_NKD):
                nc.sync.dma_start(out=xT_sb[:, k, :], in_=xr[:, k, :])
            for n in range(_NTQ):
                for m in range(4):
                    ps = psA.tile([128, 512], F32, tag="qk")
                    for k in range(_NKD):
                        nc.tensor.matmul(
                            ps,
                            lhsT=wqk_sb[:, k, m * 128:(m + 1) * 128],
                            rhs=xT_sb[:, k, n * 512:(n + 1) * 512],
                            start=(k == 0), stop=(k == _NKD - 1),
                        )
                    nc.vector.tensor_scalar_add(
                        out=qkT[:, m, n * 512:(n + 1) * 512],
                        in0=ps, scalar1=bqk_sb[:, m:m + 1],
                    )
                for t in range(4 * n, 4 * n + 4):
                    psv = psV.tile([128, 4 * HD], F32, tag="v")
                    for k in range(_NKD):
                        nc.tensor.matmul(
                            psv,
                            lhsT=xT_sb[:, k, t * 128:(t + 1) * 128],
                            rhs=wv_sb[:, k, :],
                            start=(k == 0), stop=(k == _NKD - 1),
                        )
                    nc.vector.tensor_add(
                        out=vaug[:, t, :, 0:HD],
                        in0=psv.rearrange("p (h d) -> p h d", h=HPC),
                        in1=bv_sb.rearrange("p (h d) -> p h d", h=HPC),
                    )

        # ---- Phase B+C: attention (head pairs interleaved), out_proj(j) ----
        with tc.tile_pool(name="psB", bufs=4, space="PSUM") as psB, \
             tc.tile_pool(name="expp", bufs=6) as expp, \
             tc.tile_pool(name="small", bufs=3) as small, \
             tc.tile_pool(name="dscr", bufs=2, space="DRAM") as dscr, \
             tc.tile_pool(name="outp", bufs=3) as outp:
            for j in range(_NTQ):
                ntk = 4 * j + 4
                ps_avs = []
                gath = small.tile([HPC, 512], F32, tag="gath")
                for hp in range(2):
                    h0, h1 = 2 * hp, 2 * hp + 1
                    qT0 = qkT[0:64, 2 * hp, :]
                    kT0 = qkT[0:64, 2 * hp + 1, :]
                    qT1 = qkT[64:128, 2 * hp, :]
                    kT1 = qkT[64:128, 2 * hp + 1, :]
                    av0 = psB.tile([HD + 1, 512], F32, tag="av", bufs=2)
                    av1 = psB.tile([HD + 1, 512], F32, tag="av", bufs=2)
                    for i in range(ntk):
                        a = i - 4 * j  # >= 0 on the causal staircase
                        col0 = max(a, 0) * 128
                        # keep the f32r scores matmul free dim >= 256
                        c0s = min(col0, 256)
                        tk = slice(i * 128, (i + 1) * 128)
                        tq = slice(j * 512 + c0s, (j + 1) * 512)
                        s0 = psB.tile([128, 512], F32, tag="s", bufs=4)
                        s1 = psB.tile([128, 512], F32, tag="s", bufs=4)
                        # the two heads hit disjoint PE row groups (0:64 /
                        # 64:128) and run concurrently
                        nc.tensor.matmul(s0[:, c0s:512], lhsT=kT0[:, tk],
                                         rhs=qT0[:, tq], start=True, stop=True)
                        nc.tensor.matmul(s1[:, c0s:512], lhsT=kT1[:, tk],
                                         rhs=qT1[:, tq], start=True, stop=True)
                        e0 = expp.tile([128, 512], BF16, tag="e")
                        e1 = expp.tile([128, 512], BF16, tag="e")
                        nc.scalar.activation(e0[:, col0:512], s0[:, col0:512], EXP)
                        nc.scalar.activation(e1[:, col0:512], s1[:, col0:512], EXP)
                        if a >= 0:
                            nc.vector.tensor_mul(
                                e0[:, col0:col0 + 128],
                                e0[:, col0:col0 + 128], mask_sb)
                            nc.vector.tensor_mul(
                                e1[:, col0:col0 + 128],
                                e1[:, col0:col0 + 128], mask_sb)
                        nc.tensor.matmul(
                            av0[:, col0:512], lhsT=vaug[:, i, h0, :],
                            rhs=e0[:, col0:512],
                            start=(i == 0), stop=(i == ntk - 1))
                        nc.tensor.matmul(
                            av1[:, col0:512], lhsT=vaug[:, i, h1, :],
                            rhs=e1[:, col0:512],
                            start=(i == 0), stop=(i == ntk - 1))
                    for h, av in ((h0, av0), (h1, av1)):
                        # stage the whole head result to SBUF (frees the psum
                        # bank); row 64 is sum(exp), DMA'd into the gather
                        # tile at partition h (DMA shifts partitions)
                        an = small.tile([HD + 1, 512], F32, tag="an", bufs=4)
                        nc.vector.tensor_copy(an, av)
                        ps_avs.append(an)
                        nc.sync.dma_start(out=gath[h:h + 1, :],
                                          in_=an[HD:HD + 1, :])
                # one reciprocal for all 4 heads (DVE cost is free-size bound)
                rec4 = small.tile([HPC, 512], F32, tag="rec")
                nc.vector.reciprocal(rec4, gath)
                dram4 = dscr.tile([HPC, 512], F32, tag="dt")
                nc.sync.dma_start(out=dram4, in_=rec4)
                for h in range(HPC):
                    pair, sub = h // 2, h % 2
                    sl = dram4[h:h + 1, :]
                    rb = small.tile([HD, 512], F32, tag="rb", bufs=4)
                    nc.gpsimd.dma_start(
                        out=rb,
                        in_=bass.AP(tensor=sl.tensor, offset=sl.offset,
                                    ap=[[0, HD]] + [list(p) for p in sl.ap[1:]]),
                    )
                    tmp = small.tile([HD, 512], BF16, tag="tmp", bufs=4)
                    nc.vector.tensor_mul(tmp, ps_avs[h][0:HD, :], rb)
                    nc.sync.dma_start(
                        out=aT[sub * 64:(sub + 1) * 64, pair,
                               j * 512:(j + 1) * 512],
                        in_=tmp)
                for m in range(D // 128):
                    po = psB.tile([128, 512], F32, tag="o", bufs=2)
                    for kk in range(2):
                        nc.tensor.matmul(
                            po,
                            lhsT=wo_sb[:, kk, m * 128:(m + 1) * 128],
                            rhs=aT[:, kk, j * 512:(j + 1) * 512],
                            start=(kk == 0), stop=(kk == 1),
                        )
                    ot = outp.tile([128, 512], F32, tag="ot")
                    nc.vector.tensor_copy(ot, po)
                    nc.sync.dma_start(
                        out=outT[m * 128:(m + 1) * 128, j * 512:(j + 1) * 512],
                        in_=ot)


def shard_inputs(x, W_qkv, b_qkv, W_out):
    """Host-side packing: one input dict per core."""
    x = np.asarray(x, np.float32)
    Wr = np.asarray(W_qkv, np.float32).reshape(H, 3, HD, D)
    br = np.asarray(b_qkv, np.float32).reshape(H, 3, HD)
    W_out = np.asarray(W_out, np.float32)
    scale = 1.0 / np.sqrt(HD)

    def tile_pkc(a):
        # [R, C] with R = 128*k -> [128, k, C] contiguous bf16
        r, c = a.shape
        return np.ascontiguousarray(
            a.reshape(r // 128, 128, c).transpose(1, 0, 2).astype(BF16NP))

    mask128 = np.triu(np.ones((128, 128), BF16NP))
    in_maps = []
    for core in range(NCORES):
        b, g = divmod(core, 4)
        hh = [4 * g + i for i in range(HPC)]
        # chan-tile order: q(h0)|q(h1), k(h0)|k(h1), q(h2)|q(h3), k(h2)|k(h3)
        qk_rows, qk_bias = [], []
        for p in range(2):
            h0, h1 = hh[2 * p], hh[2 * p + 1]
            qk_rows += [Wr[h0, 0] * scale, Wr[h1, 0] * scale, Wr[h0, 1], Wr[h1, 1]]
            qk_bias += [br[h0, 0] * scale, br[h1, 0] * scale, br[h0, 1], br[h1, 1]]
        wqk = np.concatenate(qk_rows, 0)          # [512, D]
        bqk = np.concatenate(qk_bias, 0)          # [512]
        wv = np.concatenate([Wr[h, 2] for h in hh], 0)   # [256, D]
        bvv = np.concatenate([br[h, 2] for h in hh], 0)  # [256]
        cols = np.concatenate([np.arange(h * HD, (h + 1) * HD) for h in hh])
        in_maps.append({
            "xr": tile_pkc(np.ascontiguousarray(x[b].T)),
            "wqk": tile_pkc(np.ascontiguousarray(wqk.T)),
            "bqk": np.ascontiguousarray(bqk.reshape(4, 128).T),
            "wv": tile_pkc(np.ascontiguousarray(wv.T)),
            "bv": np.ascontiguousarray(bvv.reshape(1, 4 * HD)),
            "wo": tile_pkc(np.ascontiguousarray(W_out[:, cols].T)),
            "mask": mask128,
            "one": np.ones((1, 1), BF16NP),
        })
    return in_maps


_NC = None


def kernel(x, mask, W_qkv, b_qkv, W_out, b_out, **run_kwargs):
    global _NC
    if _NC is None:
        _NC = build_nc()
    in_maps = shard_inputs(x, W_qkv, b_qkv, W_out)
    res = run_bass_kernel_spmd(_NC, in_maps, core_ids=list(range(NCORES)),
                               **run_kwargs)
    b_out = np.asarray(b_out, np.float64)
    outs = []
    for b in range(B):
        acc = np.zeros((D, T), np.float64)
        for g in range(4):
            acc += res.results[4 * b + g]["outT"]
        outs.append(acc.T + b_out[None, :])
    out = np.stack(outs).astype(np.float32)
    if run_kwargs:
        kernel.last_results = res
    return out


# revision 26
# speedup vs baseline: 1.4955x; 1.0342x over previous
"""Multi-head self-attention (B=2, T=2048, D=1024, H=16, causal) on 8 TRN2
NeuronCores.

Sharding: data parallel over batch (2) x tensor parallel over heads (4 groups
of 4 heads) = 8 cores. Each core computes the qkv projection for its 4 heads,
causal attention, and a partial out-projection over its heads' channels; the
host sums the 4 partials per batch and adds b_out.

Precision: x / projection weights are bf16 (input DMA + FWL weight loads),
q/k activations and the scores matmul are float32r (exp input accuracy),
exp output / v / attention-output path are bf16, all psum accumulation fp32.

Device inputs per core (host-prearranged, fully contiguous):
  xr    [128, 8, 2048] bf16   x[b]^T tiled (p, k, t): xr[p,k,t] = x[b][t, 128k+p]
  wqk   [128, 8, 512]  bf16   q/k weights, chan order
                              [q(h0)|q(h1)] [k(h0)|k(h1)] [q(h2)|q(h3)] [k(h2)|k(h3)]
                              (64 each; q pre-scaled by 1/sqrt(HD))
  wv    [128, 8, 256]  bf16   v weights [v(h0)|v(h1)|v(h2)|v(h3)]
  wo    [128, 2, 1024] bf16   W_out columns for these heads, transposed+tiled
  bqk   [128, 4] f32, bv [1, 256] f32, mask [128,128] bf16, one [1,1] bf16
Output:
  outT  [D, T] f32            partial (pre-bias) out-projection, transposed

Attention per head: scores^T tiles [tk=128, tq=512] = kT.T @ qT (K=HD=64,
f32r); the two heads of a pair sit at partitions 0:64 / 64:128 so their
scores matmuls land in disjoint PE row-groups and run concurrently. exp on
ScalarE (scores are bounded, no max-subtraction), causal handling by column
trimming + one triangular 128x128 mask multiply on the staircase block, then
out'^T [65, tq] = v_aug.T @ exp^T accumulated over tk tiles (v_aug has a
trailing ones column, so psum row 64 is sum(exp)). The 4 heads' sum(exp)
rows are gathered onto partitions 0..3 of one tile for a single batched
DVE reciprocal per stripe.
"""

import numpy as np
import ml_dtypes

import concourse.bass as bass
import concourse.tile as tile
from concourse import mybir
from concourse.bass_utils import run_bass_kernel_spmd

B, T, D, H = 2, 2048, 1024, 16
HD = D // H  # 64
NCORES = 8
HPC = 4  # heads per core
F32 = mybir.dt.float32
F32R = mybir.dt.float32r
BF16 = mybir.dt.bfloat16
EXP = mybir.ActivationFunctionType.Exp
BF16NP = ml_dtypes.bfloat16

_NTQ = T // 512  # 4 tq stripes of 512
_NTK = T // 128  # 16 tk tiles of 128
_NKD = D // 128  # 8 contraction tiles over D


def _apply_drain_patch():
    """This walrus build rejects >1 sync-wait command on a CTRL/Drain, so
    split the Tile tail-drain's waits across one drain instruction per
    pending proc."""
    import bass_rust

    if getattr(tile.TileContext, "_drain_patch_applied", False):
        return

    def _split_drain_and_barrier(self, tick_clock, wait_clock):
        nc = self.nc
        gc = tick_clock.global_clock
        NP = 27
        ticks = [gc[p] for p in range(NP)]
        for p in range(NP):
            if ticks[p] <= 0:
                continue
            partial = bass_rust.VectorClock(
                [ticks[q] if q == p else 0 for q in range(NP)]
            )
            d = nc.sync.drain()
            wait_clock.add_sem_waits(d.ins, bass_rust.ScopedClock({None: partial}))
        nc.all_engine_barrier()
        assert self.sems is not None
        popped = nc._tile_sem_poison_stack.pop()
        assert popped is self._sem_poison
        nc.clear_and_free_semaphores(list(self.sems.allocated().values()))
        nc.all_engine_barrier()

    tile.TileContext._drain_and_barrier = _split_drain_and_barrier
    tile.TileContext._drain_patch_applied = True


def _split_waits(nc):
    """This walrus build allows only one sync-wait command per instruction;
    move extra waits onto same-engine NOPs inserted right before."""
    import bass_rust

    f = nc.m.functions[0]
    ctr = 0
    for blk in f.blocks:
        insts = list(blk.instructions)
        new = []
        changed = False
        for inst in insts:
            si = getattr(inst, "sync_info", None)
            if si is not None and len(si.on_wait) > 1:
                waits = list(si.on_wait)
                for w in waits[:-1]:
                    nop = mybir.InstNoOp(name=f"wsplit-{ctr}", ins=[], outs=[])
                    ctr += 1
                    nop.engine = inst.engine
                    nop.sync_info = bass_rust.SyncInfo(on_wait=[w], on_update=[])
                    nc.register_instruction(nop, overwrite=True)
                    new.append(nop)
                inst.sync_info = bass_rust.SyncInfo(
                    on_wait=[waits[-1]], on_update=list(si.on_update))
                changed = True
            new.append(inst)
        if changed:
            blk.instructions = new


def build_nc():
    """Build the per-core Bass program (identical on all 8 cores)."""
    _apply_drain_patch()
    nc = bass.Bass("TRN2", target_bir_lowering=False, debug=False,
                   num_devices=NCORES)

    xr = nc.dram_tensor("xr", [128, _NTQ, _NKD, 512], BF16,
                        kind="ExternalInput").ap()
    wqk = nc.dram_tensor("wqk", [128, _NKD, 8 * HD], BF16,
                         kind="ExternalInput").ap()
    bqk = nc.dram_tensor("bqk", [128, 4], F32, kind="ExternalInput").ap()
    wv = nc.dram_tensor("wv", [128, _NKD, 4 * HD], BF16,
                        kind="ExternalInput").ap()
    bv = nc.dram_tensor("bv", [1, 4 * HD], F32, kind="ExternalInput").ap()
    wo = nc.dram_tensor("wo", [128, 2, D], BF16, kind="ExternalInput").ap()
    mask = nc.dram_tensor("mask", [128, 128], BF16, kind="ExternalInput").ap()
    one = nc.dram_tensor("one", [1, 1], BF16, kind="ExternalInput").ap()
    outT = nc.dram_tensor("outT", [D, T], F32, kind="ExternalOutput").ap()

    with tile.TileContext(nc) as tc:
        _emit(nc, tc, xr, wqk, bqk, wv, bv, wo, mask, one, outT)
    _split_waits(nc)
    return nc


def _emit(nc, tc, xr, wqk, bqk, wv, bv, wo, mask, one, outT):
    import contextlib

    with contextlib.ExitStack() as ctx:
        const = ctx.enter_context(tc.tile_pool(name="const", bufs=1))
        persist = ctx.enter_context(tc.tile_pool(name="persist", bufs=1))

        # weight loads on the Scalar HWDGE queue (idle at kernel start);
        # x on the Sync queue; small constants via gpsimd SWDGE
        wqk_sb = const.tile([128, _NKD, 8 * HD], BF16)
        wv_sb = const.tile([128, _NKD, 4 * HD], BF16)
        wo_sb = const.tile([128, 2, D], BF16)
        nc.scalar.dma_start(out=wqk_sb, in_=wqk)
        nc.sync.dma_start(out=wv_sb, in_=wv)
        mask_sb = const.tile([128, 128], BF16)
        nc.gpsimd.dma_start(out=mask_sb, in_=mask)
        bqk_sb = const.tile([128, 4], F32)
        nc.gpsimd.dma_start(out=bqk_sb, in_=bqk)
        bv_sb = const.tile([128, 4 * HD], F32)
        nc.gpsimd.dma_start(
            out=bv_sb,
            in_=bass.AP(tensor=bv.tensor, offset=bv.offset,
                        ap=[[0, 128], [1, 4 * HD]]),
        )

        # qkT[:, m, :]: m=0 -> q(h0)|q(h1), 1 -> k(h0)|k(h1), 2 -> q(h2)|q(h3),
        # 3 -> k(h2)|k(h3); partition p<64 is head h0/h2, p>=64 is h1/h3.
        qkT = persist.tile([128, 4, T], F32R)
        # v with a trailing ones column: [tq-part, tk-tile, head, HD+1]
        # (sum(exp) lands on psum partition 64)
        vaug = persist.tile([128, _NTK, HPC, HD + 1], BF16)
        nc.gpsimd.dma_start(
            out=vaug[:, :, :, HD:HD + 1],
            in_=bass.AP(tensor=one.tensor, offset=one.offset,
                        ap=[[0, 128], [0, _NTK * HPC], [0, 1]]),
        )
        # attention output^T, stacked [h0|h1] / [h2|h3] on partitions
        aT = persist.tile([128, 2, T], BF16)

        # ---- Phase A: qkv projections ----
        with tc.tile_pool(name="xp", bufs=1) as xp, \
             tc.tile_pool(name="psA", bufs=3, space="PSUM") as psA, \
             tc.tile_pool(name="psV", bufs=2, space="PSUM") as psV:
            xT_sb = xp.tile([128, _NKD, T], BF16)
            for n in range(_NTQ):
                eng = nc.sync if n % 2 == 0 else nc.scalar
                eng.dma_start(out=xT_sb[:, :, n * 512:(n + 1) * 512],
                              in_=xr[:, n, :, :])
            nc.scalar.dma_start(out=wo_sb, in_=wo)
            for n in range(_NTQ):
                for m in range(4):
                    ps = psA.tile([128, 512], F32, tag="qk")
                    for k in range(_NKD):
                        nc.tensor.matmul(
                            ps,
                            lhsT=wqk_sb[:, k, m * 128:(m + 1) * 128],
                            rhs=xT_sb[:, k, n * 512:(n + 1) * 512],
                            start=(k == 0), stop=(k == _NKD - 1),
                        )
                    nc.vector.tensor_scalar_add(
                        out=qkT[:, m, n * 512:(n + 1) * 512],
                        in0=ps, scalar1=bqk_sb[:, m:m + 1],
                    )
                for t in range(4 * n, 4 * n + 4):
                    psv = psV.tile([128, 4 * HD], F32, tag="v")
                    for k in range(_NKD):
                        nc.tensor.matmul(
                            psv,
                            lhsT=xT_sb[:, k, t * 128:(t + 1) * 128],
                            rhs=wv_sb[:, k, :],
                            start=(k == 0), stop=(k == _NKD - 1),
                        )
                    nc.vector.tensor_add(
                        out=vaug[:, t, :, 0:HD],
                        in0=psv.rearrange("p (h d) -> p h d", h=HPC),
                        in1=bv_sb.rearrange("p (h d) -> p h d", h=HPC),
                    )

        # ---- Phase B+C: attention (head pairs interleaved), out_proj(j) ----
        with tc.tile_pool(name="psB", bufs=4, space="PSUM") as psB, \
             tc.tile_pool(name="expp", bufs=6) as expp, \
             tc.tile_pool(name="small", bufs=3) as small, \
             tc.tile_pool(name="dscr", bufs=2, space="DRAM") as dscr, \
             tc.tile_pool(name="outp", bufs=3) as outp:
            for j in range(_NTQ):
                ntk = 4 * j + 4
                ps_avs = []
                gath = small.tile([HPC, 512], F32, tag="gath")
                for hp in range(2):
                    h0, h1 = 2 * hp, 2 * hp + 1
                    qT0 = qkT[0:64, 2 * hp, :]
                    kT0 = qkT[0:64, 2 * hp + 1, :]
                    qT1 = qkT[64:128, 2 * hp, :]
                    kT1 = qkT[64:128, 2 * hp + 1, :]
                    av0 = psB.tile([HD + 1, 512], F32, tag="av", bufs=2)
                    av1 = psB.tile([HD + 1, 512], F32, tag="av", bufs=2)
                    for i in range(ntk):
                        a = i - 4 * j  # >= 0 on the causal staircase
                        col0 = max(a, 0) * 128
                        # keep the f32r scores matmul free dim >= 256
                        c0s = min(col0, 256)
                        tk = slice(i * 128, (i + 1) * 128)
                        tq = slice(j * 512 + c0s, (j + 1) * 512)
                        s0 = psB.tile([128, 512], F32, tag="s", bufs=4)
                        s1 = psB.tile([128, 512], F32, tag="s", bufs=4)
                        # the two heads hit disjoint PE row groups (0:64 /
                        # 64:128) and run concurrently
                        nc.tensor.matmul(s0[:, c0s:512], lhsT=kT0[:, tk],
                                         rhs=qT0[:, tq], start=True, stop=True)
                        nc.tensor.matmul(s1[:, c0s:512], lhsT=kT1[:, tk],
                                         rhs=qT1[:, tq], start=True, stop=True)
                        e0 = expp.tile([128, 512], BF16, tag="e")
                        e1 = expp.tile([128, 512], BF16, tag="e")
                        nc.scalar.activation(e0[:, col0:512], s0[:, col0:512], EXP)
                        nc.scalar.activation(e1[:, col0:512], s1[:, col0:512], EXP)
                        if a >= 0:
                            nc.vector.tensor_mul(
                                e0[:, col0:col0 + 128],
                                e0[:, col0:col0 + 128], mask_sb)
                            nc.vector.tensor_mul(
                                e1[:, col0:col0 + 128],
                                e1[:, col0:col0 + 128], mask_sb)
                        nc.tensor.matmul(
                            av0[:, col0:512], lhsT=vaug[:, i, h0, :],
                            rhs=e0[:, col0:512],
                            start=(i == 0), stop=(i == ntk - 1))
                        nc.tensor.matmul(
                            av1[:, col0:512], lhsT=vaug[:, i, h1, :],
                            rhs=e1[:, col0:512],
                            start=(i == 0), stop=(i == ntk - 1))
                    for h, av in ((h0, av0), (h1, av1)):
                        # stage the whole head result to SBUF (frees the psum
                        # bank); row 64 is sum(exp), DMA'd into the gather
                        # tile at partition h (DMA shifts partitions)
                        an = small.tile([HD + 1, 512], F32, tag="an", bufs=4)
                        nc.vector.tensor_copy(an, av)
                        ps_avs.append(an)
                        nc.sync.dma_start(out=gath[h:h + 1, :],
                                          in_=an[HD:HD + 1, :])
                # one reciprocal for all 4 heads (DVE cost is free-size bound)
                rec4 = small.tile([HPC, 512], F32, tag="rec")
                nc.vector.reciprocal(rec4, gath)
                dram4 = dscr.tile([HPC, 512], F32, tag="dt")
                nc.sync.dma_start(out=dram4, in_=rec4)
                for h in range(HPC):
                    pair, sub = h // 2, h % 2
                    sl = dram4[h:h + 1, :]
                    rb = small.tile([HD, 512], F32, tag="rb", bufs=4)
                    nc.gpsimd.dma_start(
                        out=rb,
                        in_=bass.AP(tensor=sl.tensor, offset=sl.offset,
                                    ap=[[0, HD]] + [list(p) for p in sl.ap[1:]]),
                    )
                    tmp = small.tile([HD, 512], BF16, tag="tmp", bufs=4)
                    nc.vector.tensor_mul(tmp, ps_avs[h][0:HD, :], rb)
                    nc.sync.dma_start(
                        out=aT[sub * 64:(sub + 1) * 64, pair,
                               j * 512:(j + 1) * 512],
                        in_=tmp)
                for m in range(D // 128):
                    po = psB.tile([128, 512], F32, tag="o", bufs=2)
                    for kk in range(2):
                        nc.tensor.matmul(
                            po,
                            lhsT=wo_sb[:, kk, m * 128:(m + 1) * 128],
                            rhs=aT[:, kk, j * 512:(j + 1) * 512],
                            start=(kk == 0), stop=(kk == 1),
                        )
                    ot = outp.tile([128, 512], F32, tag="ot")
                    nc.vector.tensor_copy(ot, po)
                    nc.sync.dma_start(
                        out=outT[m * 128:(m + 1) * 128, j * 512:(j + 1) * 512],
                        in_=ot)


def shard_inputs(x, W_qkv, b_qkv, W_out):
    """Host-side packing: one input dict per core."""
    x = np.asarray(x, np.float32)
    Wr = np.asarray(W_qkv, np.float32).reshape(H, 3, HD, D)
    br = np.asarray(b_qkv, np.float32).reshape(H, 3, HD)
    W_out = np.asarray(W_out, np.float32)
    scale = 1.0 / np.sqrt(HD)

    def tile_pkc(a):
        # [R, C] with R = 128*k -> [128, k, C] contiguous bf16
        r, c = a.shape
        return np.ascontiguousarray(
            a.reshape(r // 128, 128, c).transpose(1, 0, 2).astype(BF16NP))

    mask128 = np.triu(np.ones((128, 128), BF16NP))
    in_maps = []
    for core in range(NCORES):
        b, g = divmod(core, 4)
        hh = [4 * g + i for i in range(HPC)]
        # chan-tile order: q(h0)|q(h1), k(h0)|k(h1), q(h2)|q(h3), k(h2)|k(h3)
        qk_rows, qk_bias = [], []
        for p in range(2):
            h0, h1 = hh[2 * p], hh[2 * p + 1]
            qk_rows += [Wr[h0, 0] * scale, Wr[h1, 0] * scale, Wr[h0, 1], Wr[h1, 1]]
            qk_bias += [br[h0, 0] * scale, br[h1, 0] * scale, br[h0, 1], br[h1, 1]]
        wqk = np.concatenate(qk_rows, 0)          # [512, D]
        bqk = np.concatenate(qk_bias, 0)          # [512]
        wv = np.concatenate([Wr[h, 2] for h in hh], 0)   # [256, D]
        bvv = np.concatenate([br[h, 2] for h in hh], 0)  # [256]
        cols = np.concatenate([np.arange(h * HD, (h + 1) * HD) for h in hh])
        in_maps.append({
            "xr": np.ascontiguousarray(
                x[b].T.reshape(_NKD, 128, _NTQ, 512)
                .transpose(1, 2, 0, 3).astype(BF16NP)),
            "wqk": tile_pkc(np.ascontiguousarray(wqk.T)),
            "bqk": np.ascontiguousarray(bqk.reshape(4, 128).T),
            "wv": tile_pkc(np.ascontiguousarray(wv.T)),
            "bv": np.ascontiguousarray(bvv.reshape(1, 4 * HD)),
            "wo": tile_pkc(np.ascontiguousarray(W_out[:, cols].T)),
            "mask": mask128,
            "one": np.ones((1, 1), BF16NP),
        })
    return in_maps


_NC = None


def kernel(x, mask, W_qkv, b_qkv, W_out, b_out, **run_kwargs):
    global _NC
    if _NC is None:
        _NC = build_nc()
    in_maps = shard_inputs(x, W_qkv, b_qkv, W_out)
    res = run_bass_kernel_spmd(_NC, in_maps, core_ids=list(range(NCORES)),
                               **run_kwargs)
    b_out = np.asarray(b_out, np.float64)
    outs = []
    for b in range(B):
        acc = np.zeros((D, T), np.float64)
        for g in range(4):
            acc += res.results[4 * b + g]["outT"]
        outs.append(acc.T + b_out[None, :])
    out = np.stack(outs).astype(np.float32)
    if run_kwargs:
        kernel.last_results = res
    return out
